# revision 25
# baseline (speedup 1.0000x reference)
"""AnisotropicEdgeFilter Trainium2 kernel (8 NeuronCores, data-parallel over edges).

Math (per edge e):
    h  = elu(pos @ W1 + b1)                       [E, 128]
    ew = (h @ W2 + b2).reshape(E, 8, 32)          per-edge filter
    out[e, o] = sum_i attr[e, i] * ew[e, i, o]    [E, 32]

Device-side restructuring:
    g = elu(x) + 1 = relu(x) + min(exp(x), 1)     (x = pos@W1+b1, b1 folded via
                                                   ones-row augmentation of pos/W1)
    ew + b2 = g @ W2 + b2'        with b2' = b2 - W2.sum(0)   (the "-1" fold)
    out = sum_i attr_i * (g @ W2)_i  + attr @ reshape(b2', (8,32))
          ^ on-device einsum           ^ "abias", precomputed on host

Layouts: hT [hidden=partition, edge=free] so the ELU'd activations are directly
the stationary weights of the W2 matmul; einsum runs in [edge=partition] layout
on VectorE: contiguous (i,o) multiply + contiguous fold-tree reduction.

Final form: CHUNK=2048 (16 sub-tiles), chunk-wide DVE ops (per-op dispatch
overhead amortized), ew produced into rotating 2-bank PSUM quarter-tiles so
the DVE multiply starts after 4 matmuls and the PE refills one quarter while
another is consumed. All inputs/outputs SBUF-resident; ~17 large DMAs per
core (attr/abias/out use a partition-major DRAM layout: one long contiguous
run per partition). Measured 338.7us on 8 cores (baseline 3710us).
"""

import os
import sys

import numpy as np

sys.path.insert(0, "/opt/trn_rl_repo")

import ml_dtypes  # noqa: E402

E = 500000
IN_SIZE = 8
POS_SIZE = 6
HIDDEN = 128
OUT_SIZE = 32
N_CORES = 8
CHUNK = 2048           # edges per inner chunk (16 sub-tiles of 128)
N_CHUNKS = 31
N_SUB = CHUNK // 128   # 8
E_LOC = CHUNK * N_CHUNKS      # 63488 edges per core
E_PAD = E_LOC * N_CORES       # 507904

PIECE = 4              # chunks per post-DMA piece
N_PIECES = (N_CHUNKS + PIECE - 1) // PIECE
OUT_GROUPS = [(i, min(i + 4, N_CHUNKS)) for i in range(0, N_CHUNKS, 4)]
WARMUP_MMS = 48

_BF16 = ml_dtypes.bfloat16

_COMPILED = {}


def _build_nc(n_chunks=N_CHUNKS):
    import concourse.bass as bass
    import concourse.tile as tile
    from concourse import bacc, mybir

    e_loc = CHUNK * n_chunks
    dt = mybir.dt
    nc = bacc.Bacc(
        "TRN2",
        target_bir_lowering=False,
        debug=False,
        num_devices=N_CORES,
    )

    post_d = nc.dram_tensor("post", [POS_SIZE + 1, e_loc], dt.bfloat16, kind="ExternalInput")
    attr_d = nc.dram_tensor("attr", [128, n_chunks, N_SUB, IN_SIZE], dt.bfloat16, kind="ExternalInput")
    abias_d = nc.dram_tensor("abias", [128, n_chunks, N_SUB, OUT_SIZE], dt.bfloat16, kind="ExternalInput")
    w1_d = nc.dram_tensor("w1aug", [POS_SIZE + 1, HIDDEN], dt.bfloat16, kind="ExternalInput")
    w2_d = nc.dram_tensor("w2", [HIDDEN, IN_SIZE * OUT_SIZE], dt.bfloat16, kind="ExternalInput")
    out_d = nc.dram_tensor("out", [128, n_chunks, N_SUB, OUT_SIZE], dt.bfloat16, kind="ExternalOutput")
    wu_d = nc.dram_tensor("wu", [128, 1], dt.float32, kind="ExternalOutput")

    ACT = mybir.ActivationFunctionType
    ALU = mybir.AluOpType

    with tile.TileContext(nc) as tc:
        with (
            tc.tile_pool(name="wpool", bufs=1) as wpool,
            tc.tile_pool(name="postp", bufs=3) as postp,
            tc.tile_pool(name="outg", bufs=1) as outg,
            tc.tile_pool(name="hps", bufs=1, space="PSUM") as hps_pool,
            tc.tile_pool(name="ewps", bufs=2, space="PSUM") as ewps_pool,
            tc.tile_pool(name="work", bufs=3) as work,
        ):
            # (PE clock is pinned at 1.2 GHz in this environment — HAM never
            # opens even under a 14us continuous matmul burst, so no warm-up.)
            wu_out = wpool.tile([128, 1], dt.float32, name="wu_out")
            nc.gpsimd.memset(wu_out[:], 0.0)
            nc.sync.dma_start(wu_d.ap(), wu_out[:])

            w1_sb = wpool.tile([POS_SIZE + 1, HIDDEN], dt.bfloat16)
            nc.sync.dma_start(w1_sb[:], w1_d.ap())

            out_tiles = [
                outg.tile(
                    [128, g1 - g0, N_SUB, OUT_SIZE],
                    dt.bfloat16,
                    name=f"out{gi}",
                    tag=f"out{gi}",
                )
                for gi, (g0, g1) in enumerate(OUT_GROUPS)
            ]

            post_ap = post_d.ap()
            out_ap = out_d.ap()

            # post pieces: a tiny first piece so chunk 0 starts ASAP
            piece_starts = [0, 1] + list(range(1 + PIECE, n_chunks, PIECE))
            piece_of = {}
            for j, a in enumerate(piece_starts):
                b = piece_starts[j + 1] if j + 1 < len(piece_starts) else n_chunks
                for cc in range(a, b):
                    piece_of[cc] = (j, a, b)
            post_tiles = [None] * len(piece_starts)
            post_ap_early = post_d.ap()
            for j in range(min(3, len(piece_starts))):
                a = piece_starts[j]
                b = piece_starts[j + 1] if j + 1 < len(piece_starts) else n_chunks
                pt = postp.tile(
                    [POS_SIZE + 1, PIECE * CHUNK], dt.bfloat16, tag="post",
                    name="post_pre",
                )
                nc.sync.dma_start(
                    pt[:, : (b - a) * CHUNK], post_ap_early[:, a * CHUNK : b * CHUNK]
                )
                post_tiles[j] = pt

            w2_sb = wpool.tile([HIDDEN, IN_SIZE * OUT_SIZE], dt.bfloat16)
            nc.sync.dma_start(w2_sb[:], w2_d.ap())
            attr_all = wpool.tile([128, n_chunks, N_SUB, IN_SIZE], dt.bfloat16)
            nc.scalar.dma_start(attr_all[:], attr_d.ap())
            abias_all = wpool.tile([128, n_chunks, N_SUB, OUT_SIZE], dt.bfloat16)
            nc.scalar.dma_start(abias_all[:], abias_d.ap())
            grp_of = {}
            for gidx, (a, b) in enumerate(OUT_GROUPS):
                for cc in range(a, b):
                    grp_of[cc] = gidx

            def emit_folds(cc, prod):
                # fold tree of chunk cc, deferred one chunk: it runs on the
                # DVE while the PE fills chunk cc+1's first ew quarter
                g0, g1 = OUT_GROUPS[grp_of[cc]]
                outt = out_tiles[grp_of[cc]]
                t1 = work.tile(
                    [128, N_SUB, 4, OUT_SIZE], dt.bfloat16, tag="t1", name="t1"
                )
                nc.vector.tensor_add(t1[:], prod[:, :, 0:4, :], prod[:, :, 4:8, :])
                t2 = work.tile(
                    [128, N_SUB, 2, OUT_SIZE], dt.bfloat16, tag="t2", name="t2"
                )
                nc.vector.tensor_add(t2[:], t1[:, :, 0:2, :], t1[:, :, 2:4, :])
                t3 = work.tile(
                    [128, N_SUB, OUT_SIZE], dt.bfloat16, tag="t3", name="t3"
                )
                nc.vector.tensor_add(t3[:], t2[:, :, 0, :], t2[:, :, 1, :])
                nc.vector.tensor_add(
                    outt[:, cc - g0, :, :], t3[:], abias_all[:, cc, :, :]
                )
                if cc == g1 - 1:
                    nc.sync.dma_start(out_ap[:, g0:g1, :, :], outt[:])

            pending = None
            for c in range(n_chunks):
                j, a, b = piece_of[c]
                if c == a and post_tiles[j] is None:
                    pt = postp.tile([POS_SIZE + 1, PIECE * CHUNK], dt.bfloat16, tag="post")
                    nc.sync.dma_start(
                        pt[:, : (b - a) * CHUNK], post_ap[:, a * CHUNK : b * CHUNK]
                    )
                    post_tiles[j] = pt

                pos_sb = post_tiles[j]
                off = (c - a) * CHUNK

                # x = posT_aug.T @ W1aug  ->  hT psum [hidden=128, CHUNK]
                hps = hps_pool.tile([HIDDEN, CHUNK], dt.float32, tag="hps")
                for h in range(CHUNK // 512):
                    nc.tensor.matmul(
                        hps[:, h * 512 : (h + 1) * 512],
                        w1_sb[:],
                        pos_sb[:, off + h * 512 : off + (h + 1) * 512],
                        start=True,
                        stop=True,
                    )

                # g = relu(x) + min(exp(x), 1)   (= elu(x) + 1)
                e_sb = work.tile([HIDDEN, CHUNK], dt.bfloat16, tag="exp")
                nc.scalar.activation(e_sb[:], hps[:], ACT.Exp)
                r_sb = work.tile([HIDDEN, CHUNK], dt.bfloat16, tag="relu")
                nc.scalar.activation(r_sb[:], hps[:], ACT.Relu)
                v_sb = work.tile([HIDDEN, CHUNK], dt.bfloat16, tag="vmin")
                nc.vector.tensor_scalar_min(v_sb[:], e_sb[:], 1.0)
                g_sb = work.tile([HIDDEN, CHUNK], dt.bfloat16, tag="g")
                nc.vector.tensor_add(g_sb[:], v_sb[:], r_sb[:])

                # previous chunk's fold tree fills the DVE window while the
                # PE produces this chunk's first ew quarter
                if pending is not None:
                    emit_folds(*pending)

                # ew in rotating 2-bank PSUM quarter-tiles (bufs=2) so the DVE
                # multiply starts after 4 matmuls instead of 16, and the PE
                # refills one quarter while another is being consumed
                prod = work.tile(
                    [128, N_SUB, IN_SIZE, OUT_SIZE], dt.bfloat16, tag="prod", bufs=2
                )
                half = N_SUB // 4
                for hb in range(4):
                    ewp = ewps_pool.tile(
                        [128, half, IN_SIZE * OUT_SIZE], dt.float32, tag="ewp", name="ewp"
                    )
                    for k in range(half):
                        s = hb * half + k
                        nc.tensor.matmul(
                            ewp[:, k, :],
                            g_sb[:, s * 128 : (s + 1) * 128],
                            w2_sb[:],
                            start=True,
                            stop=True,
                        )
                    # prod[p, s, i, o] = ew[p, s, i*32+o] * attr[p, c, s, i]
                    # (ew stays in its natural contiguous (i, o) layout; the
                    # i-reduction is a contiguous fold tree in 2x DVE mode)
                    ew_v = ewp[:].rearrange(
                        "p s (i o) -> p s i o", i=IN_SIZE, o=OUT_SIZE
                    )
                    at_v = (
                        attr_all[:, c, hb * half : (hb + 1) * half, :]
                        .unsqueeze(3)
                        .broadcast_to([128, half, IN_SIZE, OUT_SIZE])
                    )
                    nc.vector.tensor_tensor(
                        prod[:, hb * half : (hb + 1) * half], ew_v, at_v, op=ALU.mult
                    )
                pending = (c, prod)

            emit_folds(*pending)

    nc.compile()
    return nc


def _get_compiled(n_chunks=N_CHUNKS):
    if n_chunks not in _COMPILED:
        _COMPILED[n_chunks] = _build_nc(n_chunks)
    return _COMPILED[n_chunks]


def _prep_shards(edge_attr, edge_pos, W1, b1, W2, b2):
    """Host-side prep: pad, fold biases, transpose, tile, cast to bf16."""
    ea = np.asarray(edge_attr, dtype=np.float32)
    ep = np.asarray(edge_pos, dtype=np.float32)
    W1 = np.asarray(W1, dtype=np.float32)
    b1 = np.asarray(b1, dtype=np.float32)
    W2 = np.asarray(W2, dtype=np.float32)
    b2 = np.asarray(b2, dtype=np.float32)

    n = ea.shape[0]
    pad = E_PAD - n
    ea_p = np.pad(ea, ((0, pad), (0, 0)))
    ep_p = np.pad(ep, ((0, pad), (0, 0)))

    # b2' = b2 - W2.sum(0); abias = attr @ reshape(b2', (8, 32))
    b2p = b2 - W2.sum(axis=0)
    abias = ea_p @ b2p.reshape(IN_SIZE, OUT_SIZE)  # [E_PAD, 32] f32

    w1aug = np.concatenate([W1, b1[None, :]], axis=0).astype(_BF16)  # [7, 128]
    w2_bf = W2.astype(_BF16)

    in_maps = []
    for i in range(N_CORES):
        sl = slice(i * E_LOC, (i + 1) * E_LOC)
        pos_sh = ep_p[sl]  # [E_LOC, 6]
        post = np.empty((POS_SIZE + 1, E_LOC), dtype=_BF16)
        post[:POS_SIZE] = pos_sh.T.astype(_BF16)
        post[POS_SIZE] = _BF16(1.0)
        # edge (c, s, p) -> attr_d[p, c, s, :]
        attr_sh = (
            ea_p[sl].reshape(N_CHUNKS, N_SUB, 128, IN_SIZE).transpose(2, 0, 1, 3)
        ).astype(_BF16)  # [128, C, S, 8]
        abias_sh = (
            abias[sl].reshape(N_CHUNKS, N_SUB, 128, OUT_SIZE).transpose(2, 0, 1, 3)
        ).astype(_BF16)  # [128, C, S, 32]
        in_maps.append(
            {
                "post": np.ascontiguousarray(post),
                "attr": np.ascontiguousarray(attr_sh),
                "abias": np.ascontiguousarray(abias_sh),
                "w1aug": w1aug,
                "w2": w2_bf,
            }
        )
    return in_maps


def kernel(**inputs) -> np.ndarray:
    from concourse.bass_utils import run_bass_kernel_spmd

    n = inputs["edge_attr"].shape[0]
    in_maps = _prep_shards(
        inputs["edge_attr"], inputs["edge_pos"],
        inputs["W1"], inputs["b1"], inputs["W2"], inputs["b2"],
    )
    nc = _get_compiled()
    res = run_bass_kernel_spmd(nc, in_maps, core_ids=list(range(N_CORES)))
    outs = []
    for i in range(N_CORES):
        o = np.asarray(res.results[i]["out"])  # [128, C, S, 32] bf16
        o = o.astype(np.float32).transpose(1, 2, 0, 3).reshape(E_LOC, OUT_SIZE)
        outs.append(o)
    full = np.concatenate(outs, axis=0)[:n]
    return np.ascontiguousarray(full)


# revision 30
# speedup vs baseline: 1.0361x; 1.0361x over previous
"""AnisotropicEdgeFilter Trainium2 kernel (8 NeuronCores, data-parallel over edges).

Math (per edge e):
    h  = elu(pos @ W1 + b1)                       [E, 128]
    ew = (h @ W2 + b2).reshape(E, 8, 32)          per-edge filter
    out[e, o] = sum_i attr[e, i] * ew[e, i, o]    [E, 32]

Device-side restructuring:
    g = elu(x) + 1 = relu(x) + min(exp(x), 1)     (x = pos@W1+b1, b1 folded via
                                                   ones-row augmentation of pos/W1)
    ew + b2 = g @ W2 + b2'        with b2' = b2 - W2.sum(0)   (the "-1" fold)
    out = sum_i attr_i * (g @ W2)_i  + attr @ reshape(b2', (8,32))
          ^ on-device einsum           ^ "abias", precomputed on host

Layouts: hT [hidden=partition, edge=free] so the ELU'd activations are directly
the stationary weights of the W2 matmul; einsum runs in [edge=partition] layout
on VectorE: contiguous (i,o) multiply + contiguous fold-tree reduction.

Final form: CHUNK=2048 (16 sub-tiles), chunk-wide DVE ops (per-op dispatch
overhead amortized), ew produced into rotating 2-bank PSUM quarter-tiles so
the DVE multiply starts after 4 matmuls and the PE refills one quarter while
another is consumed. All inputs/outputs SBUF-resident; ~17 large DMAs per
core (attr/abias/out use a partition-major DRAM layout: one long contiguous
run per partition); fold tree runs one chunk deferred to fill the PE-fill
window; inputs split across both HWDGE rings. Measured 331.9us on 8 cores
(baseline 3710us).
"""

import os
import sys

import numpy as np

sys.path.insert(0, "/opt/trn_rl_repo")

import ml_dtypes  # noqa: E402

E = 500000
IN_SIZE = 8
POS_SIZE = 6
HIDDEN = 128
OUT_SIZE = 32
N_CORES = 8
CHUNK = 2048           # edges per inner chunk (16 sub-tiles of 128)
N_CHUNKS = 31
N_SUB = CHUNK // 128   # 8
E_LOC = CHUNK * N_CHUNKS      # 63488 edges per core
E_PAD = E_LOC * N_CORES       # 507904

PIECE = 4              # chunks per post-DMA piece
N_PIECES = (N_CHUNKS + PIECE - 1) // PIECE
OUT_GROUPS = [(i, min(i + 4, N_CHUNKS)) for i in range(0, N_CHUNKS, 4)]
WARMUP_MMS = 48

_BF16 = ml_dtypes.bfloat16

_COMPILED = {}


def _build_nc(n_chunks=N_CHUNKS):
    import concourse.bass as bass
    import concourse.tile as tile
    from concourse import bacc, mybir

    e_loc = CHUNK * n_chunks
    dt = mybir.dt
    nc = bacc.Bacc(
        "TRN2",
        target_bir_lowering=False,
        debug=False,
        num_devices=N_CORES,
    )

    post_d = nc.dram_tensor("post", [POS_SIZE + 1, e_loc], dt.bfloat16, kind="ExternalInput")
    attr_d = nc.dram_tensor("attr", [128, n_chunks, N_SUB, IN_SIZE], dt.bfloat16, kind="ExternalInput")
    abias_d = nc.dram_tensor("abias", [128, n_chunks, N_SUB, OUT_SIZE], dt.bfloat16, kind="ExternalInput")
    w1_d = nc.dram_tensor("w1aug", [POS_SIZE + 1, HIDDEN], dt.bfloat16, kind="ExternalInput")
    w2_d = nc.dram_tensor("w2", [HIDDEN, IN_SIZE * OUT_SIZE], dt.bfloat16, kind="ExternalInput")
    out_d = nc.dram_tensor("out", [128, n_chunks, N_SUB, OUT_SIZE], dt.bfloat16, kind="ExternalOutput")
    wu_d = nc.dram_tensor("wu", [128, 1], dt.float32, kind="ExternalOutput")

    ACT = mybir.ActivationFunctionType
    ALU = mybir.AluOpType

    with tile.TileContext(nc) as tc:
        with (
            tc.tile_pool(name="wpool", bufs=1) as wpool,
            tc.tile_pool(name="postp", bufs=3) as postp,
            tc.tile_pool(name="outg", bufs=1) as outg,
            tc.tile_pool(name="hps", bufs=1, space="PSUM") as hps_pool,
            tc.tile_pool(name="ewps", bufs=2, space="PSUM") as ewps_pool,
            tc.tile_pool(name="work", bufs=3) as work,
        ):
            # (PE clock is pinned at 1.2 GHz in this environment — HAM never
            # opens even under a 14us continuous matmul burst, so no warm-up.)
            w1_sb = wpool.tile([POS_SIZE + 1, HIDDEN], dt.bfloat16)
            nc.sync.dma_start(w1_sb[:], w1_d.ap())

            out_tiles = [
                outg.tile(
                    [128, g1 - g0, N_SUB, OUT_SIZE],
                    dt.bfloat16,
                    name=f"out{gi}",
                    tag=f"out{gi}",
                )
                for gi, (g0, g1) in enumerate(OUT_GROUPS)
            ]

            post_ap = post_d.ap()
            out_ap = out_d.ap()

            # post pieces: a tiny first piece so chunk 0 starts ASAP
            piece_starts = [0, 1] + list(range(1 + PIECE, n_chunks, PIECE))
            piece_of = {}
            for j, a in enumerate(piece_starts):
                b = piece_starts[j + 1] if j + 1 < len(piece_starts) else n_chunks
                for cc in range(a, b):
                    piece_of[cc] = (j, a, b)
            post_tiles = [None] * len(piece_starts)
            for j in range(min(3, len(piece_starts))):
                a = piece_starts[j]
                b = piece_starts[j + 1] if j + 1 < len(piece_starts) else n_chunks
                pt = postp.tile(
                    [POS_SIZE + 1, PIECE * CHUNK], dt.bfloat16, tag="post",
                    name="post_pre",
                )
                nc.sync.dma_start(
                    pt[:, : (b - a) * CHUNK], post_ap[:, a * CHUNK : b * CHUNK]
                )
                post_tiles[j] = pt

            w2_sb = wpool.tile([HIDDEN, IN_SIZE * OUT_SIZE], dt.bfloat16)
            nc.sync.dma_start(w2_sb[:], w2_d.ap())
            attr_all = wpool.tile([128, n_chunks, N_SUB, IN_SIZE], dt.bfloat16)
            nc.scalar.dma_start(attr_all[:], attr_d.ap())
            # abias split per out-group: chunk 0's folds only need the first
            # ~512KB slice, not the whole 4MB transfer
            abias_all = wpool.tile([128, n_chunks, N_SUB, OUT_SIZE], dt.bfloat16)
            abias_ap = abias_d.ap()
            for (ga, gb) in OUT_GROUPS:
                nc.scalar.dma_start(
                    abias_all[:, ga:gb, :, :], abias_ap[:, ga:gb, :, :]
                )
            grp_of = {}
            for gidx, (a, b) in enumerate(OUT_GROUPS):
                for cc in range(a, b):
                    grp_of[cc] = gidx

            def emit_folds(cc, prod):
                # fold tree of chunk cc, deferred one chunk: it runs on the
                # DVE while the PE fills chunk cc+1's first ew quarter
                g0, g1 = OUT_GROUPS[grp_of[cc]]
                outt = out_tiles[grp_of[cc]]
                t1 = work.tile(
                    [128, N_SUB, 4, OUT_SIZE], dt.bfloat16, tag="t1", name="t1"
                )
                nc.vector.tensor_add(t1[:], prod[:, :, 0:4, :], prod[:, :, 4:8, :])
                t2 = work.tile(
                    [128, N_SUB, 2, OUT_SIZE], dt.bfloat16, tag="t2", name="t2"
                )
                nc.vector.tensor_add(t2[:], t1[:, :, 0:2, :], t1[:, :, 2:4, :])
                t3 = work.tile(
                    [128, N_SUB, OUT_SIZE], dt.bfloat16, tag="t3", name="t3"
                )
                nc.vector.tensor_add(t3[:], t2[:, :, 0, :], t2[:, :, 1, :])
                nc.vector.tensor_add(
                    outt[:, cc - g0, :, :], t3[:], abias_all[:, cc, :, :]
                )
                if cc == g1 - 1:
                    nc.sync.dma_start(out_ap[:, g0:g1, :, :], outt[:])

            pending = None
            for c in range(n_chunks):
                j, a, b = piece_of[c]
                if c == a and post_tiles[j] is None:
                    pt = postp.tile([POS_SIZE + 1, PIECE * CHUNK], dt.bfloat16, tag="post")
                    nc.sync.dma_start(
                        pt[:, : (b - a) * CHUNK], post_ap[:, a * CHUNK : b * CHUNK]
                    )
                    post_tiles[j] = pt

                pos_sb = post_tiles[j]
                off = (c - a) * CHUNK

                # x = posT_aug.T @ W1aug  ->  hT psum [hidden=128, CHUNK]
                hps = hps_pool.tile([HIDDEN, CHUNK], dt.float32, tag="hps")
                for h in range(CHUNK // 512):
                    nc.tensor.matmul(
                        hps[:, h * 512 : (h + 1) * 512],
                        w1_sb[:],
                        pos_sb[:, off + h * 512 : off + (h + 1) * 512],
                        start=True,
                        stop=True,
                    )

                # g = relu(x) + min(exp(x), 1)   (= elu(x) + 1)
                e_sb = work.tile([HIDDEN, CHUNK], dt.bfloat16, tag="exp")
                nc.scalar.activation(e_sb[:], hps[:], ACT.Exp)
                r_sb = work.tile([HIDDEN, CHUNK], dt.bfloat16, tag="relu")
                nc.scalar.activation(r_sb[:], hps[:], ACT.Relu)
                v_sb = work.tile([HIDDEN, CHUNK], dt.bfloat16, tag="vmin")
                nc.vector.tensor_scalar_min(v_sb[:], e_sb[:], 1.0)
                g_sb = work.tile([HIDDEN, CHUNK], dt.bfloat16, tag="g")
                nc.vector.tensor_add(g_sb[:], v_sb[:], r_sb[:])

                # previous chunk's fold tree fills the DVE window while the
                # PE produces this chunk's first ew quarter
                if pending is not None:
                    emit_folds(*pending)

                # ew in rotating 2-bank PSUM quarter-tiles (bufs=2) so the DVE
                # multiply starts after 4 matmuls instead of 16, and the PE
                # refills one quarter while another is being consumed
                prod = work.tile(
                    [128, N_SUB, IN_SIZE, OUT_SIZE], dt.bfloat16, tag="prod", bufs=2
                )
                half = N_SUB // 4
                for hb in range(4):
                    ewp = ewps_pool.tile(
                        [128, half, IN_SIZE * OUT_SIZE], dt.float32, tag="ewp", name="ewp"
                    )
                    for k in range(half):
                        s = hb * half + k
                        nc.tensor.matmul(
                            ewp[:, k, :],
                            g_sb[:, s * 128 : (s + 1) * 128],
                            w2_sb[:],
                            start=True,
                            stop=True,
                        )
                    # prod[p, s, i, o] = ew[p, s, i*32+o] * attr[p, c, s, i]
                    # (ew stays in its natural contiguous (i, o) layout; the
                    # i-reduction is a contiguous fold tree in 2x DVE mode)
                    ew_v = ewp[:].rearrange(
                        "p s (i o) -> p s i o", i=IN_SIZE, o=OUT_SIZE
                    )
                    at_v = (
                        attr_all[:, c, hb * half : (hb + 1) * half, :]
                        .unsqueeze(3)
                        .broadcast_to([128, half, IN_SIZE, OUT_SIZE])
                    )
                    nc.vector.tensor_tensor(
                        prod[:, hb * half : (hb + 1) * half], ew_v, at_v, op=ALU.mult
                    )
                pending = (c, prod)

            emit_folds(*pending)

            wu_out = wpool.tile([128, 1], dt.float32, name="wu_out")
            nc.gpsimd.memset(wu_out[:], 0.0)
            nc.sync.dma_start(wu_d.ap(), wu_out[:])

            wu_out = wpool.tile([128, 1], dt.float32, name="wu_out")
            nc.gpsimd.memset(wu_out[:], 0.0)
            nc.sync.dma_start(wu_d.ap(), wu_out[:])

    nc.compile()
    return nc


def _get_compiled(n_chunks=N_CHUNKS):
    if n_chunks not in _COMPILED:
        _COMPILED[n_chunks] = _build_nc(n_chunks)
    return _COMPILED[n_chunks]


def _prep_shards(edge_attr, edge_pos, W1, b1, W2, b2):
    """Host-side prep: pad, fold biases, transpose, tile, cast to bf16."""
    ea = np.asarray(edge_attr, dtype=np.float32)
    ep = np.asarray(edge_pos, dtype=np.float32)
    W1 = np.asarray(W1, dtype=np.float32)
    b1 = np.asarray(b1, dtype=np.float32)
    W2 = np.asarray(W2, dtype=np.float32)
    b2 = np.asarray(b2, dtype=np.float32)

    n = ea.shape[0]
    pad = E_PAD - n
    ea_p = np.pad(ea, ((0, pad), (0, 0)))
    ep_p = np.pad(ep, ((0, pad), (0, 0)))

    # b2' = b2 - W2.sum(0); abias = attr @ reshape(b2', (8, 32))
    b2p = b2 - W2.sum(axis=0)
    abias = ea_p @ b2p.reshape(IN_SIZE, OUT_SIZE)  # [E_PAD, 32] f32

    w1aug = np.concatenate([W1, b1[None, :]], axis=0).astype(_BF16)  # [7, 128]
    w2_bf = W2.astype(_BF16)

    in_maps = []
    for i in range(N_CORES):
        sl = slice(i * E_LOC, (i + 1) * E_LOC)
        pos_sh = ep_p[sl]  # [E_LOC, 6]
        post = np.empty((POS_SIZE + 1, E_LOC), dtype=_BF16)
        post[:POS_SIZE] = pos_sh.T.astype(_BF16)
        post[POS_SIZE] = _BF16(1.0)
        # edge (c, s, p) -> attr_d[p, c, s, :]
        attr_sh = (
            ea_p[sl].reshape(N_CHUNKS, N_SUB, 128, IN_SIZE).transpose(2, 0, 1, 3)
        ).astype(_BF16)  # [128, C, S, 8]
        abias_sh = (
            abias[sl].reshape(N_CHUNKS, N_SUB, 128, OUT_SIZE).transpose(2, 0, 1, 3)
        ).astype(_BF16)  # [128, C, S, 32]
        in_maps.append(
            {
                "post": np.ascontiguousarray(post),
                "attr": np.ascontiguousarray(attr_sh),
                "abias": np.ascontiguousarray(abias_sh),
                "w1aug": w1aug,
                "w2": w2_bf,
            }
        )
    return in_maps


def kernel(**inputs) -> np.ndarray:
    from concourse.bass_utils import run_bass_kernel_spmd

    n = inputs["edge_attr"].shape[0]
    in_maps = _prep_shards(
        inputs["edge_attr"], inputs["edge_pos"],
        inputs["W1"], inputs["b1"], inputs["W2"], inputs["b2"],
    )
    nc = _get_compiled()
    res = run_bass_kernel_spmd(nc, in_maps, core_ids=list(range(N_CORES)))
    outs = []
    for i in range(N_CORES):
        o = np.asarray(res.results[i]["out"])  # [128, C, S, 32] bf16
        o = o.astype(np.float32).transpose(1, 2, 0, 3).reshape(E_LOC, OUT_SIZE)
        outs.append(o)
    full = np.concatenate(outs, axis=0)[:n]
    return np.ascontiguousarray(full)
